# revision 20
# baseline (speedup 1.0000x reference)
"""Trainium2 Bass kernel for nn_CNN4CH (3x stride-2 conv -> GAP -> MLP -> 3x3 Procrustes).

Strategy (pure data parallelism, 4 samples per core on 8 cores):
  - Host: pad x, build conv1 im2col layout x100[(c,dy,dx), yo, g] covering 2x2
    output-pixel blocks (K=100, stride-4 windows), cast everything to bf16.
  - Device per sample:
      conv1: single K=100 matmul per row-pair -> PSUM[(yph,xph,c32), 304]
             -> ReLU+bias (ScalarE) into 4-phase SBUF layout h1ph[128, 89, 305].
      conv2: 4 shift-matmuls (K=128) over the phase layout -> PSUM[64, 304]
             -> ReLU+bias (VectorE) into y-phase-split h2ph[128, 45, 306].
      conv3: 6 matmuls (3 dx-taps x {K=128 full, K=64 odd-row}) with stride-2
             rhs APs -> PSUM[128, 456] -> fused ReLU+bias+row-sum (accum_out).
      GAP + FC1(relu) + FC2 on device -> r[9, 4] fp32 per core.
  - Host: gather r -> [32, 3, 3], SVD -> closest-rotation projection (exact ref math).
"""

import numpy as np
import ml_dtypes
from contextlib import ExitStack

BF16 = ml_dtypes.bfloat16

B, CIN, H, W = 32, 4, 352, 1216
NCORES = 8
SPC = B // NCORES            # samples per core
H1, W1 = 176, 608            # conv1 out
H2, W2 = 88, 304             # conv2 out
H3, W3 = 44, 152             # conv3 out
YO, G = H1 // 2, W1 // 2     # conv1 row-pair / col-pair grid = 88 x 304
K1 = 100                     # c(4) * dy(5) * dx(5)
CHUNK = 22                   # conv1 yo rows per DMA chunk (88 = 4*22)
G3 = 3                       # conv3 output rows per matmul group
NG3 = (H3 + G3 - 1) // G3    # 15 groups (14x3 + 1x2)
POOLN = H3 * W3              # 6688 spatial positions averaged

_CACHE = {}


def _build_device(repeat=1):
    if ("nc", repeat) in _CACHE:
        return _CACHE[("nc", repeat)]
    import concourse.bass as bass
    import concourse.bacc as bacc
    import concourse.tile as tile
    import concourse.mybir as mybir

    dt = mybir.dt
    AF = mybir.ActivationFunctionType
    ALU = mybir.AluOpType

    nc = bacc.Bacc(
        "TRN2", target_bir_lowering=False, debug=False,
        enable_asserts=False, num_devices=NCORES,
    )

    # ---- DRAM I/O ----
    x100_d = nc.dram_tensor("x100", [SPC, K1, YO, W2], dt.bfloat16, kind="ExternalInput")
    w1l_d = nc.dram_tensor("w1l", [K1, 128], dt.bfloat16, kind="ExternalInput")
    w2l_d = nc.dram_tensor("w2l", [128, 3 * 64], dt.bfloat16, kind="ExternalInput")
    w3a_d = nc.dram_tensor("w3a", [128, 3 * 128], dt.bfloat16, kind="ExternalInput")
    w3b_d = nc.dram_tensor("w3b", [64, 3 * 128], dt.bfloat16, kind="ExternalInput")
    b1_d = nc.dram_tensor("b1", [128, 1], dt.float32, kind="ExternalInput")
    b2_d = nc.dram_tensor("b2", [64, 1], dt.float32, kind="ExternalInput")
    b3_d = nc.dram_tensor("b3", [128, 1], dt.float32, kind="ExternalInput")
    hbar_d = nc.dram_tensor("hbar_out", [128, SPC], dt.float32, kind="ExternalOutput")

    # ---- persistent SBUF ----
    h1ph_t = nc.alloc_sbuf_tensor("h1ph", [128, YO + 1, W2 + 1], dt.bfloat16)
    h2ph_t = nc.alloc_sbuf_tensor("h2ph", [128, H3 + 1, W2 + 2], dt.bfloat16)
    w1l_t = nc.alloc_sbuf_tensor("w1l_s", [K1, 128], dt.bfloat16)
    w2l_t = nc.alloc_sbuf_tensor("w2l_s", [128, 3 * 64], dt.bfloat16)
    w3a_t = nc.alloc_sbuf_tensor("w3a_s", [128, 3 * 128], dt.bfloat16)
    w3b_t = nc.alloc_sbuf_tensor("w3b_s", [128, 3 * 128], dt.bfloat16)
    b1_t = nc.alloc_sbuf_tensor("b1_s", [128, 1], dt.float32)
    b2_t = nc.alloc_sbuf_tensor("b2_s", [64, 1], dt.float32)
    b3_t = nc.alloc_sbuf_tensor("b3_s", [128, 1], dt.float32)
    h3sums_t = nc.alloc_sbuf_tensor("h3sums", [128, NG3], dt.float32)
    hbar_t = nc.alloc_sbuf_tensor("hbar", [128, SPC], dt.float32)

    h1ph = h1ph_t.ap()
    h2ph = h2ph_t.ap()

    SHIFTS = [(0, 0), (0, -1), (-1, 0), (-1, -1)]

    with TileCtx(tile, nc) as (ctx, tc):
        # weight/bias loads
        nc.sync.dma_start(w1l_t.ap()[:], w1l_d.ap()[:])
        nc.sync.dma_start(w2l_t.ap()[:], w2l_d.ap()[:])
        nc.sync.dma_start(w3a_t.ap()[:], w3a_d.ap()[:])
        nc.sync.dma_start(w3b_t.ap()[64:128, :], w3b_d.ap()[:])
        nc.sync.dma_start(b1_t.ap()[:], b1_d.ap()[:])
        nc.sync.dma_start(b2_t.ap()[:], b2_d.ap()[:])
        nc.sync.dma_start(b3_t.ap()[:], b3_d.ap()[:])
        # zero halos (row 0 / col 0 / col 305 never written afterwards)
        nc.gpsimd.memset(h1ph[:], 0.0)
        nc.gpsimd.memset(h2ph[:], 0.0)

        xpool = ctx.enter_context(tc.tile_pool(name="xch", bufs=3))
        l2pool = ctx.enter_context(tc.tile_pool(name="l2", bufs=2))
        p1 = ctx.enter_context(tc.tile_pool(name="p1", bufs=2, space="PSUM"))
        p2 = ctx.enter_context(tc.tile_pool(name="p2", bufs=3, space="PSUM"))
        p3 = ctx.enter_context(tc.tile_pool(name="p3", bufs=2, space="PSUM"))
        scr = ctx.enter_context(tc.tile_pool(name="h3scr", bufs=2))
        L2CH = 11  # conv2 rows per L2 chunk (88 = 8*11)

        x100 = x100_d.ap()
        for s in [si for _ in range(repeat) for si in range(SPC)]:
            # ---- conv1 ----
            for ch in range(YO // CHUNK):
                xt = xpool.tile([K1, CHUNK, W2], dt.bfloat16, tag="xch")
                nc.sync.dma_start(xt[:], x100[s, :, ch * CHUNK:(ch + 1) * CHUNK, :])
                for r in range(CHUNK):
                    yo = ch * CHUNK + r
                    ps = p1.tile([128, W2], dt.float32, tag="p1")
                    nc.tensor.matmul(ps[:], w1l_t.ap()[:], xt[:, r, :],
                                     start=True, stop=True)
                    nc.scalar.activation(h1ph[:, yo + 1, 1:W2 + 1], ps[:],
                                         AF.Relu, bias=b1_t.ap()[:])
            # ---- conv2: mm1 = shift(0,0) on h1ph; mm2 = K=128 L2 re-pack of
            #      4 of the 5 remaining taps; mm3 = K=32 last tap (oo@(-1,-1)).
            for chk in range(H2 // L2CH):
                c0 = chk * L2CH
                l2 = l2pool.tile([128, L2CH, W2], dt.bfloat16, tag="l2")
                # eo@(0,-1) | oe@(-1,0) | oo@(0,-1) | oo@(-1,0)
                nc.gpsimd.dma_start(l2[0:32, :, :], h1ph[32:64, 1 + c0:1 + c0 + L2CH, 0:W2])
                nc.gpsimd.dma_start(l2[32:64, :, :], h1ph[64:96, c0:c0 + L2CH, 1:W2 + 1])
                nc.gpsimd.dma_start(l2[64:96, :, :], h1ph[96:128, 1 + c0:1 + c0 + L2CH, 0:W2])
                nc.gpsimd.dma_start(l2[96:128, :, :], h1ph[96:128, c0:c0 + L2CH, 1:W2 + 1])
                for r in range(L2CH):
                    yo2 = c0 + r
                    ps2 = p2.tile([64, W2], dt.float32, tag="p2")
                    nc.tensor.matmul(ps2[:], w2l_t.ap()[:, 0:64],
                                     h1ph[:, 1 + yo2, 1:1 + W2],
                                     start=True, stop=False)
                    nc.tensor.matmul(ps2[:], w2l_t.ap()[:, 64:128],
                                     l2[:, r, :], start=False, stop=False)
                    nc.tensor.matmul(ps2[:], w2l_t.ap()[96:128, 128:192],
                                     h1ph[96:128, yo2, 0:W2],
                                     start=False, stop=True,
                                     tile_position=(96, 0))
                    po = 64 * (yo2 % 2)
                    nc.vector.tensor_scalar(
                        out=h2ph[po:po + 64, 1 + yo2 // 2, 1:W2 + 1],
                        in0=ps2[:], scalar1=b2_t.ap()[:], scalar2=0.0,
                        op0=ALU.add, op1=ALU.max,
                    )
            # ---- conv3 + pooled row-sums ----
            for g in range(NG3):
                y0 = g * G3
                rows = min(G3, H3 - y0)
                n3 = rows * W3
                ps3 = p3.tile([128, G3 * W3], dt.float32, tag="p3")
                for dxp in range(3):
                    nc.tensor.matmul(
                        ps3[:, 0:n3],
                        w3a_t.ap()[:, dxp * 128:(dxp + 1) * 128],
                        h2ph[:, 1 + y0:1 + y0 + rows, dxp:dxp + 2 * W3:2],
                        start=(dxp == 0), stop=False,
                    )
                    nc.tensor.matmul(
                        ps3[:, 0:n3],
                        w3b_t.ap()[64:128, dxp * 128:(dxp + 1) * 128],
                        h2ph[64:128, y0:y0 + rows, dxp:dxp + 2 * W3:2],
                        start=False, stop=(dxp == 2),
                    )
                h3s = scr.tile([128, G3 * W3], dt.bfloat16, tag="h3scr")
                nc.scalar.activation(h3s[:, 0:n3], ps3[:, 0:n3], AF.Relu,
                                     bias=b3_t.ap()[:],
                                     accum_out=h3sums_t.ap()[:, g:g + 1])
            # ---- GAP for this sample ----
            nc.vector.tensor_reduce(hbar_t.ap()[:, s:s + 1], h3sums_t.ap()[:],
                                    axis=mybir.AxisListType.X, op=ALU.add)
        nc.sync.dma_start(hbar_d.ap()[:], hbar_t.ap()[:])

    nc.compile()
    _CACHE[("nc", repeat)] = nc
    return nc


class TileCtx:
    """ExitStack + TileContext combined context manager."""

    def __init__(self, tile_mod, nc):
        self.tile_mod = tile_mod
        self.nc = nc

    def __enter__(self):
        self.ctx = ExitStack()
        self.tc = self.tile_mod.TileContext(self.nc)
        self.tc.__enter__()
        return self.ctx, self.tc

    def __exit__(self, *exc):
        try:
            self.ctx.close()
        finally:
            return self.tc.__exit__(*exc)


def _host_prepare(x, wc1, bc1, wc2, bc2, wc3, bc3, wl1, bl1, wl2, bl2):
    """Build per-core input maps (im2col'd x + weight layouts, bf16)."""
    xp = np.pad(np.asarray(x, dtype=np.float32), ((0, 0), (0, 0), (1, 1), (1, 1)))
    sN, sC, sH, sW = xp.strides
    # x100[b, c, dy, dx, yo, g] = xp[b, c, 4*yo+dy, 4*g+dx]
    win = np.lib.stride_tricks.as_strided(
        xp, (B, CIN, 5, 5, YO, G), (sN, sC, sH, sW, 4 * sH, 4 * sW))
    x100 = np.ascontiguousarray(win.reshape(B, K1, YO, G)).astype(BF16)

    # conv1 weights: lhsT [100, 128]; m = r*64 + j*32 + co; p = c*25 + dy*5 + dx
    w1l = np.zeros((K1, 128), np.float32)
    for r in range(2):
        for j in range(2):
            for dyp in range(3):
                for dxp in range(3):
                    dy, dx = 2 * r + dyp, 2 * j + dxp
                    for c in range(CIN):
                        p = c * 25 + dy * 5 + dx
                        w1l[p, r * 64 + j * 32 + np.arange(32)] = wc1[:, c, dyp, dxp]

    # conv2 weights [128, 3*64]:
    #  block0 (mm1, shift (0,0)): p=(yph,xph,c) -> tap (dy'=1+yph, dx'=1+xph)
    #  block1 (mm2, L2 re-pack): rows 0-31 eo@(0,-1)->(1,0); 32-63 oe@(-1,0)->(0,1);
    #                            64-95 oo@(0,-1)->(2,0); 96-127 oo@(-1,0)->(0,2)
    #  block2 (mm3): rows 96-127 oo@(-1,-1)->(0,0)
    w2l = np.zeros((128, 3 * 64), np.float32)
    for yph in range(2):
        for xph in range(2):
            for c in range(32):
                w2l[yph * 64 + xph * 32 + c, 0:64] = wc2[:, c, 1 + yph, 1 + xph]
    for c in range(32):
        w2l[c, 64:128] = wc2[:, c, 1, 0]
        w2l[32 + c, 64:128] = wc2[:, c, 0, 1]
        w2l[64 + c, 64:128] = wc2[:, c, 2, 0]
        w2l[96 + c, 64:128] = wc2[:, c, 0, 2]
        w2l[96 + c, 128:192] = wc2[:, c, 0, 0]

    # conv3: A [128, 3*128] (yph0 -> dy'=1, yph1 -> dy'=2); B [64, 3*128] (dy'=0)
    w3a = np.zeros((128, 3 * 128), np.float32)
    w3b = np.zeros((64, 3 * 128), np.float32)
    for dxp in range(3):
        for c in range(64):
            w3a[c, dxp * 128:(dxp + 1) * 128] = wc3[:, c, 1, dxp]
            w3a[64 + c, dxp * 128:(dxp + 1) * 128] = wc3[:, c, 2, dxp]
            w3b[c, dxp * 128:(dxp + 1) * 128] = wc3[:, c, 0, dxp]

    b1 = np.tile(np.asarray(bc1, np.float32), 4).reshape(128, 1)
    b2 = np.asarray(bc2, np.float32).reshape(64, 1)
    b3 = np.asarray(bc3, np.float32).reshape(128, 1)

    shared = {
        "w1l": w1l.astype(BF16), "w2l": w2l.astype(BF16),
        "w3a": w3a.astype(BF16), "w3b": w3b.astype(BF16),
        "b1": b1, "b2": b2, "b3": b3,
    }
    in_maps = []
    for core in range(NCORES):
        m = dict(shared)
        m["x100"] = np.ascontiguousarray(x100[core * SPC:(core + 1) * SPC])
        in_maps.append(m)
    return in_maps


def _procrustes(r):
    R = r.reshape(-1, 3, 3).astype(np.float32)
    U, _, Vh = np.linalg.svd(R)
    det = np.linalg.det(U @ Vh)
    U[:, :, -1] *= np.sign(det)[:, None]
    return (U @ Vh).astype(np.float32)


def _host_tail(hbar, wl1, bl1, wl2, bl2):
    """hbar: [B, 128] pooled sums (not yet divided by POOLN)."""
    h = hbar.astype(np.float32) / float(POOLN)
    h = np.maximum(h @ np.asarray(wl1, np.float32).T + np.asarray(bl1, np.float32), 0)
    r = h @ np.asarray(wl2, np.float32).T + np.asarray(bl2, np.float32)
    return _procrustes(r)


def kernel(**inputs):
    from concourse.bass_utils import run_bass_kernel_spmd
    nc = _build_device()
    in_maps = _host_prepare(**inputs)
    res = run_bass_kernel_spmd(nc, in_maps, list(range(NCORES)))
    hbar = np.concatenate(
        [res.results[i]["hbar_out"].T for i in range(NCORES)], axis=0)
    return _host_tail(hbar, inputs["wl1"], inputs["bl1"], inputs["wl2"], inputs["bl2"])


if __name__ == "__main__":
    d = np.load("inputs.npz")
    out = kernel(**{k: d[k] for k in d.files})
    exp = np.load("expected.npy")
    err = np.abs(out - exp).max()
    print("absmax err:", err, "rel:", err / np.abs(exp).max())


# revision 29
# speedup vs baseline: 2.2903x; 2.2903x over previous
"""Trainium2 Bass kernel for nn_CNN4CH (3x stride-2 conv -> GAP -> MLP -> 3x3 Procrustes).

Strategy (pure data parallelism, 4 samples per core on 8 cores):
  - Host: pad x, build conv1 im2col layout x100[(c,dy,dx), yo, g] covering 2x2
    output-pixel blocks (K=100, stride-4 windows), cast everything to bf16.
  - Device per sample:
      conv1: single K=100 matmul per row-pair -> PSUM[(yph,xph,c32), 304]
             -> ReLU+bias (ScalarE) into 4-phase SBUF layout h1ph[128, 89, 305].
      conv2: 4 shift-matmuls (K=128) over the phase layout -> PSUM[64, 304]
             -> ReLU+bias (VectorE) into y-phase-split h2ph[128, 45, 306].
      conv3: 6 matmuls (3 dx-taps x {K=128 full, K=64 odd-row}) with stride-2
             rhs APs -> PSUM[128, 456] -> fused ReLU+bias+row-sum (accum_out).
      GAP + FC1(relu) + FC2 on device -> r[9, 4] fp32 per core.
  - Host: gather r -> [32, 3, 3], SVD -> closest-rotation projection (exact ref math).
"""

import numpy as np
import ml_dtypes
from contextlib import ExitStack

BF16 = ml_dtypes.bfloat16

B, CIN, H, W = 32, 4, 352, 1216
NCORES = 8
SPC = B // NCORES            # samples per core
H1, W1 = 176, 608            # conv1 out
H2, W2 = 88, 304             # conv2 out
H3, W3 = 44, 152             # conv3 out
YO, G = H1 // 2, W1 // 2     # conv1 row-pair / col-pair grid = 88 x 304
K1 = 100                     # c(4) * dy(5) * dx(5)
CHUNK = 22                   # conv1 yo rows per DMA chunk (88 = 4*22)
G3 = 3                       # conv3 output rows per matmul group
NG3 = (H3 + G3 - 1) // G3    # 15 groups (14x3 + 1x2)
POOLN = H3 * W3              # 6688 spatial positions averaged

_CACHE = {}


def _build_device(repeat=1):
    if ("nc", repeat) in _CACHE:
        return _CACHE[("nc", repeat)]
    import concourse.bass as bass
    import concourse.bacc as bacc
    import concourse.tile as tile
    import concourse.mybir as mybir

    dt = mybir.dt
    AF = mybir.ActivationFunctionType
    ALU = mybir.AluOpType

    nc = bacc.Bacc(
        "TRN2", target_bir_lowering=False, debug=False,
        enable_asserts=False, num_devices=NCORES,
    )

    # ---- DRAM I/O ----
    x100_d = nc.dram_tensor("x100", [SPC, K1, YO, W2], dt.bfloat16, kind="ExternalInput")
    w1l_d = nc.dram_tensor("w1l", [K1, 128], dt.bfloat16, kind="ExternalInput")
    w2l_d = nc.dram_tensor("w2l", [128, 4 * 64], dt.bfloat16, kind="ExternalInput")
    w3a_d = nc.dram_tensor("w3a", [128, 3 * 128], dt.bfloat16, kind="ExternalInput")
    w3b_d = nc.dram_tensor("w3b", [64, 3 * 128], dt.bfloat16, kind="ExternalInput")
    b1_d = nc.dram_tensor("b1", [128, 1], dt.float32, kind="ExternalInput")
    b2_d = nc.dram_tensor("b2", [128, 1], dt.float32, kind="ExternalInput")
    b3_d = nc.dram_tensor("b3", [128, 1], dt.float32, kind="ExternalInput")
    hbar_d = nc.dram_tensor("hbar_out", [128, SPC], dt.float32, kind="ExternalOutput")

    # ---- persistent SBUF ----
    h1ph_t = nc.alloc_sbuf_tensor("h1ph", [128, YO + 1, W2 + 1], dt.bfloat16)
    h2ph_t = nc.alloc_sbuf_tensor("h2ph", [128, H3 + 1, W2 + 2], dt.bfloat16)
    w1l_t = nc.alloc_sbuf_tensor("w1l_s", [K1, 128], dt.bfloat16)
    w2l_t = nc.alloc_sbuf_tensor("w2l_s", [128, 4 * 64], dt.bfloat16)
    w3a_t = nc.alloc_sbuf_tensor("w3a_s", [128, 3 * 128], dt.bfloat16)
    w3b_t = nc.alloc_sbuf_tensor("w3b_s", [128, 3 * 128], dt.bfloat16)
    b1_t = nc.alloc_sbuf_tensor("b1_s", [128, 1], dt.float32)
    b2_t = nc.alloc_sbuf_tensor("b2_s", [128, 1], dt.float32)
    b3_t = nc.alloc_sbuf_tensor("b3_s", [128, 1], dt.float32)
    h3sums_t = nc.alloc_sbuf_tensor("h3sums", [128, NG3], dt.float32)
    hbar_t = nc.alloc_sbuf_tensor("hbar", [128, SPC], dt.float32)

    h1ph = h1ph_t.ap()
    h2ph = h2ph_t.ap()

    SHIFTS = [(0, 0), (0, -1), (-1, 0), (-1, -1)]

    with TileCtx(tile, nc) as (ctx, tc):
        # weight/bias loads
        nc.sync.dma_start(w1l_t.ap()[:], w1l_d.ap()[:])
        nc.sync.dma_start(w2l_t.ap()[:], w2l_d.ap()[:])
        nc.sync.dma_start(w3a_t.ap()[:], w3a_d.ap()[:])
        nc.sync.dma_start(w3b_t.ap()[64:128, :], w3b_d.ap()[:])
        nc.sync.dma_start(b1_t.ap()[:], b1_d.ap()[:])
        nc.sync.dma_start(b2_t.ap()[:], b2_d.ap()[:])
        nc.sync.dma_start(b3_t.ap()[:], b3_d.ap()[:])
        # zero halos (row 0 / col 0 / col 305 never written afterwards)
        nc.gpsimd.memset(h1ph[:], 0.0)
        nc.gpsimd.memset(h2ph[:], 0.0)

        xpool = ctx.enter_context(tc.tile_pool(name="xch", bufs=3))
        p1 = ctx.enter_context(tc.tile_pool(name="p1", bufs=2, space="PSUM"))
        p2 = ctx.enter_context(tc.tile_pool(name="p2", bufs=3, space="PSUM"))
        p3 = ctx.enter_context(tc.tile_pool(name="p3", bufs=2, space="PSUM"))
        scr = ctx.enter_context(tc.tile_pool(name="h3scr", bufs=2))

        x100 = x100_d.ap()
        for s in [si for _ in range(repeat) for si in range(SPC)]:
            # ---- conv1 ----
            for ch in range(YO // CHUNK):
                xt = xpool.tile([K1, CHUNK, W2], dt.bfloat16, tag="xch")
                nc.sync.dma_start(xt[:], x100[s, :, ch * CHUNK:(ch + 1) * CHUNK, :])
                for r in range(CHUNK):
                    yo = ch * CHUNK + r
                    ps = p1.tile([128, W2], dt.float32, tag="p1")
                    nc.tensor.matmul(ps[:], w1l_t.ap()[:], xt[:, r, :],
                                     start=True, stop=True)
                    nc.scalar.activation(h1ph[:, yo + 1, 1:W2 + 1], ps[:],
                                         AF.Relu, bias=b1_t.ap()[:])
            # ---- conv2: 4 shift-matmuls; two output rows concurrently in the
            #      two 64-col halves of the PE array (col-tiling). One PSUM
            #      bank holds both rows; single accumulation group.
            for yp in range(H2 // 2):
                Yo = 2 * yp
                ps2 = p2.tile([128, W2], dt.float32, tag="p2")
                for k, (sy, sx) in enumerate(SHIFTS):
                    nc.tensor.matmul(
                        ps2[0:64, :], w2l_t.ap()[:, k * 64:(k + 1) * 64],
                        h1ph[:, 1 + Yo + sy, 1 + sx:1 + sx + W2],
                        start=(k == 0), stop=False, tile_position=(0, 0),
                        skip_group_check=True,
                    )
                    nc.tensor.matmul(
                        ps2[64:128, :], w2l_t.ap()[:, k * 64:(k + 1) * 64],
                        h1ph[:, 2 + Yo + sy, 1 + sx:1 + sx + W2],
                        start=(k == 0), stop=(k == 3), tile_position=(0, 64),
                        skip_group_check=True,
                    )
                # rows Yo (yph0 -> partitions 0-63) and Yo+1 (yph1 -> 64-127)
                # both land at phase-row 1+yp: one fused bias+relu copy.
                nc.vector.tensor_scalar(
                    out=h2ph[:, 1 + yp, 1:W2 + 1],
                    in0=ps2[:], scalar1=b2_t.ap()[:], scalar2=0.0,
                    op0=ALU.add, op1=ALU.max,
                )
            # ---- conv3 + pooled row-sums ----
            for g in range(NG3):
                y0 = g * G3
                rows = min(G3, H3 - y0)
                n3 = rows * W3
                ps3 = p3.tile([128, G3 * W3], dt.float32, tag="p3")
                for dxp in range(3):
                    nc.tensor.matmul(
                        ps3[:, 0:n3],
                        w3a_t.ap()[:, dxp * 128:(dxp + 1) * 128],
                        h2ph[:, 1 + y0:1 + y0 + rows, dxp:dxp + 2 * W3:2],
                        start=(dxp == 0), stop=False,
                    )
                    nc.tensor.matmul(
                        ps3[:, 0:n3],
                        w3b_t.ap()[64:128, dxp * 128:(dxp + 1) * 128],
                        h2ph[64:128, y0:y0 + rows, dxp:dxp + 2 * W3:2],
                        start=False, stop=(dxp == 2),
                    )
                h3s = scr.tile([128, G3 * W3], dt.bfloat16, tag="h3scr")
                nc.scalar.activation(h3s[:, 0:n3], ps3[:, 0:n3], AF.Relu,
                                     bias=b3_t.ap()[:],
                                     accum_out=h3sums_t.ap()[:, g:g + 1])
            # ---- GAP for this sample ----
            nc.vector.tensor_reduce(hbar_t.ap()[:, s:s + 1], h3sums_t.ap()[:],
                                    axis=mybir.AxisListType.X, op=ALU.add)
        nc.sync.dma_start(hbar_d.ap()[:], hbar_t.ap()[:])

    nc.compile()
    _CACHE[("nc", repeat)] = nc
    return nc


class TileCtx:
    """ExitStack + TileContext combined context manager."""

    def __init__(self, tile_mod, nc):
        self.tile_mod = tile_mod
        self.nc = nc

    def __enter__(self):
        self.ctx = ExitStack()
        self.tc = self.tile_mod.TileContext(self.nc)
        self.tc.__enter__()
        return self.ctx, self.tc

    def __exit__(self, *exc):
        try:
            self.ctx.close()
        finally:
            return self.tc.__exit__(*exc)


def _host_prepare(x, wc1, bc1, wc2, bc2, wc3, bc3, wl1, bl1, wl2, bl2):
    """Build per-core input maps (im2col'd x + weight layouts, bf16)."""
    xp = np.pad(np.asarray(x, dtype=np.float32), ((0, 0), (0, 0), (1, 1), (1, 1)))
    sN, sC, sH, sW = xp.strides
    # x100[b, c, dy, dx, yo, g] = xp[b, c, 4*yo+dy, 4*g+dx]
    win = np.lib.stride_tricks.as_strided(
        xp, (B, CIN, 5, 5, YO, G), (sN, sC, sH, sW, 4 * sH, 4 * sW))
    x100 = np.ascontiguousarray(win.reshape(B, K1, YO, G)).astype(BF16)

    # conv1 weights: lhsT [100, 128]; m = r*64 + j*32 + co; p = c*25 + dy*5 + dx
    w1l = np.zeros((K1, 128), np.float32)
    for r in range(2):
        for j in range(2):
            for dyp in range(3):
                for dxp in range(3):
                    dy, dx = 2 * r + dyp, 2 * j + dxp
                    for c in range(CIN):
                        p = c * 25 + dy * 5 + dx
                        w1l[p, r * 64 + j * 32 + np.arange(32)] = wc1[:, c, dyp, dxp]

    # conv2 shift weights: [128, 4*64]; partition p = yph*64 + xph*32 + c
    SHIFTS = [(0, 0), (0, -1), (-1, 0), (-1, -1)]
    w2l = np.zeros((128, 4 * 64), np.float32)
    for k, (sy, sx) in enumerate(SHIFTS):
        for yph in range(2):
            for xph in range(2):
                if sy == 0:
                    dyp = 1 if yph == 0 else 2
                elif yph == 1:
                    dyp = 0
                else:
                    continue
                if sx == 0:
                    dxp = 1 if xph == 0 else 2
                elif xph == 1:
                    dxp = 0
                else:
                    continue
                for c in range(32):
                    w2l[yph * 64 + xph * 32 + c, k * 64:(k + 1) * 64] = wc2[:, c, dyp, dxp]

    # conv3: A [128, 3*128] (yph0 -> dy'=1, yph1 -> dy'=2); B [64, 3*128] (dy'=0)
    w3a = np.zeros((128, 3 * 128), np.float32)
    w3b = np.zeros((64, 3 * 128), np.float32)
    for dxp in range(3):
        for c in range(64):
            w3a[c, dxp * 128:(dxp + 1) * 128] = wc3[:, c, 1, dxp]
            w3a[64 + c, dxp * 128:(dxp + 1) * 128] = wc3[:, c, 2, dxp]
            w3b[c, dxp * 128:(dxp + 1) * 128] = wc3[:, c, 0, dxp]

    b1 = np.tile(np.asarray(bc1, np.float32), 4).reshape(128, 1)
    b2 = np.tile(np.asarray(bc2, np.float32), 2).reshape(128, 1)
    b3 = np.asarray(bc3, np.float32).reshape(128, 1)

    shared = {
        "w1l": w1l.astype(BF16), "w2l": w2l.astype(BF16),
        "w3a": w3a.astype(BF16), "w3b": w3b.astype(BF16),
        "b1": b1, "b2": b2, "b3": b3,
    }
    in_maps = []
    for core in range(NCORES):
        m = dict(shared)
        m["x100"] = np.ascontiguousarray(x100[core * SPC:(core + 1) * SPC])
        in_maps.append(m)
    return in_maps


def _procrustes(r):
    R = r.reshape(-1, 3, 3).astype(np.float32)
    U, _, Vh = np.linalg.svd(R)
    det = np.linalg.det(U @ Vh)
    U[:, :, -1] *= np.sign(det)[:, None]
    return (U @ Vh).astype(np.float32)


def _host_tail(hbar, wl1, bl1, wl2, bl2):
    """hbar: [B, 128] pooled sums (not yet divided by POOLN)."""
    h = hbar.astype(np.float32) / float(POOLN)
    h = np.maximum(h @ np.asarray(wl1, np.float32).T + np.asarray(bl1, np.float32), 0)
    r = h @ np.asarray(wl2, np.float32).T + np.asarray(bl2, np.float32)
    return _procrustes(r)


def kernel(**inputs):
    from concourse.bass_utils import run_bass_kernel_spmd
    nc = _build_device()
    in_maps = _host_prepare(**inputs)
    res = run_bass_kernel_spmd(nc, in_maps, list(range(NCORES)))
    hbar = np.concatenate(
        [res.results[i]["hbar_out"].T for i in range(NCORES)], axis=0)
    return _host_tail(hbar, inputs["wl1"], inputs["bl1"], inputs["wl2"], inputs["bl2"])


if __name__ == "__main__":
    d = np.load("inputs.npz")
    out = kernel(**{k: d[k] for k in d.files})
    exp = np.load("expected.npy")
    err = np.abs(out - exp).max()
    print("absmax err:", err, "rel:", err / np.abs(exp).max())


# revision 31
# speedup vs baseline: 4.4045x; 1.9231x over previous
"""Trainium2 Bass kernel for nn_CNN4CH (3x stride-2 conv -> GAP -> MLP -> 3x3 Procrustes).

Strategy (pure data parallelism, 4 samples per core on 8 cores):
  - Host: pad x, build conv1 im2col layout x100[(c,dy,dx), yo, g] covering 2x2
    output-pixel blocks (K=100, stride-4 windows), cast everything to bf16.
  - Device per sample:
      conv1: single K=100 matmul per row-pair -> PSUM[(yph,xph,c32), 304]
             -> ReLU+bias (ScalarE) into 4-phase SBUF layout h1ph[128, 89, 305].
      conv2: 4 shift-matmuls (K=128) over the phase layout -> PSUM[64, 304]
             -> ReLU+bias (VectorE) into y-phase-split h2ph[128, 45, 306].
      conv3: 6 matmuls (3 dx-taps x {K=128 full, K=64 odd-row}) with stride-2
             rhs APs -> PSUM[128, 456] -> fused ReLU+bias+row-sum (accum_out).
      GAP + FC1(relu) + FC2 on device -> r[9, 4] fp32 per core.
  - Host: gather r -> [32, 3, 3], SVD -> closest-rotation projection (exact ref math).
"""

import numpy as np
import ml_dtypes
from contextlib import ExitStack

BF16 = ml_dtypes.bfloat16

B, CIN, H, W = 32, 4, 352, 1216
NCORES = 8
SPC = B // NCORES            # samples per core
H1, W1 = 176, 608            # conv1 out
H2, W2 = 88, 304             # conv2 out
H3, W3 = 44, 152             # conv3 out
YO, G = H1 // 2, W1 // 2     # conv1 row-pair / col-pair grid = 88 x 304
K1 = 100                     # c(4) * dy(5) * dx(5)
CHUNK = 22                   # conv1 yo rows per DMA chunk (88 = 4*22)
G3 = 3                       # conv3 output rows per matmul group
NG3 = (H3 + G3 - 1) // G3    # 15 groups (14x3 + 1x2)
POOLN = H3 * W3              # 6688 spatial positions averaged

_CACHE = {}


def _build_device(repeat=1):
    if ("nc", repeat) in _CACHE:
        return _CACHE[("nc", repeat)]
    import concourse.bass as bass
    import concourse.bacc as bacc
    import concourse.tile as tile
    import concourse.mybir as mybir

    dt = mybir.dt
    AF = mybir.ActivationFunctionType
    ALU = mybir.AluOpType

    nc = bacc.Bacc(
        "TRN2", target_bir_lowering=False, debug=False,
        enable_asserts=False, num_devices=NCORES,
    )

    # ---- DRAM I/O ----
    x100_d = nc.dram_tensor("x100", [SPC, K1, YO, W2], dt.bfloat16, kind="ExternalInput")
    w1l_d = nc.dram_tensor("w1l", [K1, 128], dt.bfloat16, kind="ExternalInput")
    w2l_d = nc.dram_tensor("w2l", [128, 4 * 64], dt.bfloat16, kind="ExternalInput")
    w3a_d = nc.dram_tensor("w3a", [128, 3 * 128], dt.bfloat16, kind="ExternalInput")
    w3b_d = nc.dram_tensor("w3b", [64, 3 * 128], dt.bfloat16, kind="ExternalInput")
    b1_d = nc.dram_tensor("b1", [128, 1], dt.float32, kind="ExternalInput")
    b2_d = nc.dram_tensor("b2", [128, 1], dt.float32, kind="ExternalInput")
    b3_d = nc.dram_tensor("b3", [128, 1], dt.float32, kind="ExternalInput")
    hbar_d = nc.dram_tensor("hbar_out", [128, SPC], dt.float32, kind="ExternalOutput")

    # ---- persistent SBUF ----
    h1ph_t = nc.alloc_sbuf_tensor("h1ph", [128, YO + 1, W2 + 1], dt.bfloat16)
    h2ph_t = nc.alloc_sbuf_tensor("h2ph", [128, H3 + 1, W2 + 2], dt.bfloat16)
    w1l_t = nc.alloc_sbuf_tensor("w1l_s", [K1, 128], dt.bfloat16)
    w2l_t = nc.alloc_sbuf_tensor("w2l_s", [128, 4 * 64], dt.bfloat16)
    w3a_t = nc.alloc_sbuf_tensor("w3a_s", [128, 3 * 128], dt.bfloat16)
    w3b_t = nc.alloc_sbuf_tensor("w3b_s", [128, 3 * 128], dt.bfloat16)
    b1_t = nc.alloc_sbuf_tensor("b1_s", [128, 1], dt.float32)
    b2_t = nc.alloc_sbuf_tensor("b2_s", [128, 1], dt.float32)
    b3_t = nc.alloc_sbuf_tensor("b3_s", [128, 1], dt.float32)
    h3sums_t = nc.alloc_sbuf_tensor("h3sums", [128, NG3], dt.float32)
    hbar_t = nc.alloc_sbuf_tensor("hbar", [128, SPC], dt.float32)

    h1ph = h1ph_t.ap()
    h2ph = h2ph_t.ap()

    SHIFTS = [(0, 0), (0, -1), (-1, 0), (-1, -1)]

    with TileCtx(tile, nc) as (ctx, tc):
        # weight/bias loads
        nc.sync.dma_start(w1l_t.ap()[:], w1l_d.ap()[:])
        nc.sync.dma_start(w2l_t.ap()[:], w2l_d.ap()[:])
        nc.sync.dma_start(w3a_t.ap()[:], w3a_d.ap()[:])
        nc.sync.dma_start(w3b_t.ap()[64:128, :], w3b_d.ap()[:])
        nc.sync.dma_start(b1_t.ap()[:], b1_d.ap()[:])
        nc.sync.dma_start(b2_t.ap()[:], b2_d.ap()[:])
        nc.sync.dma_start(b3_t.ap()[:], b3_d.ap()[:])
        # zero halos (only borders are ever read as padding)
        nc.gpsimd.memset(h1ph[:, 0, :], 0.0)
        nc.gpsimd.memset(h1ph[:, :, 0:1], 0.0)
        nc.gpsimd.memset(h2ph[:, 0, :], 0.0)
        nc.gpsimd.memset(h2ph[:, :, 0:1], 0.0)

        xpool = ctx.enter_context(tc.tile_pool(name="xch", bufs=3))
        p1 = ctx.enter_context(tc.tile_pool(name="p1", bufs=2, space="PSUM"))
        p2 = ctx.enter_context(tc.tile_pool(name="p2", bufs=3, space="PSUM"))
        p3 = ctx.enter_context(tc.tile_pool(name="p3", bufs=2, space="PSUM"))
        scr = ctx.enter_context(tc.tile_pool(name="h3scr", bufs=2))

        x100 = x100_d.ap()
        for s in [si for _ in range(repeat) for si in range(SPC)]:
            # ---- conv1 ----
            for ch in range(YO // CHUNK):
                xt = xpool.tile([K1, CHUNK, W2], dt.bfloat16, tag="xch")
                nc.sync.dma_start(xt[:], x100[s, :, ch * CHUNK:(ch + 1) * CHUNK, :])
                for r in range(CHUNK):
                    yo = ch * CHUNK + r
                    ps = p1.tile([128, W2], dt.float32, tag="p1")
                    nc.tensor.matmul(ps[:], w1l_t.ap()[:], xt[:, r, :],
                                     start=True, stop=True)
                    # alternate ACT/DVE to balance engine load
                    if yo % 2 == 0:
                        nc.scalar.activation(h1ph[:, yo + 1, 1:W2 + 1], ps[:],
                                             AF.Relu, bias=b1_t.ap()[:])
                    else:
                        nc.vector.tensor_scalar(
                            out=h1ph[:, yo + 1, 1:W2 + 1], in0=ps[:],
                            scalar1=b1_t.ap()[:], scalar2=0.0,
                            op0=ALU.add, op1=ALU.max,
                        )
            # ---- conv2: 4 shift-matmuls; two output rows concurrently in the
            #      two 64-col halves of the PE array (col-tiling). One PSUM
            #      bank holds both rows; single accumulation group.
            for yp in range(H2 // 2):
                Yo = 2 * yp
                ps2 = p2.tile([128, W2], dt.float32, tag="p2")
                for k, (sy, sx) in enumerate(SHIFTS):
                    nc.tensor.matmul(
                        ps2[0:64, :], w2l_t.ap()[:, k * 64:(k + 1) * 64],
                        h1ph[:, 1 + Yo + sy, 1 + sx:1 + sx + W2],
                        start=(k == 0), stop=False, tile_position=(0, 0),
                        skip_group_check=True,
                    )
                    nc.tensor.matmul(
                        ps2[64:128, :], w2l_t.ap()[:, k * 64:(k + 1) * 64],
                        h1ph[:, 2 + Yo + sy, 1 + sx:1 + sx + W2],
                        start=(k == 0), stop=(k == 3), tile_position=(0, 64),
                        skip_group_check=True,
                    )
                # rows Yo (yph0 -> partitions 0-63) and Yo+1 (yph1 -> 64-127)
                # both land at phase-row 1+yp: one fused bias+relu copy.
                nc.vector.tensor_scalar(
                    out=h2ph[:, 1 + yp, 1:W2 + 1],
                    in0=ps2[:], scalar1=b2_t.ap()[:], scalar2=0.0,
                    op0=ALU.add, op1=ALU.max,
                )
            # ---- conv3 + pooled row-sums ----
            for g in range(NG3):
                y0 = g * G3
                rows = min(G3, H3 - y0)
                n3 = rows * W3
                ps3 = p3.tile([128, G3 * W3], dt.float32, tag="p3")
                for dxp in range(3):
                    nc.tensor.matmul(
                        ps3[:, 0:n3],
                        w3a_t.ap()[:, dxp * 128:(dxp + 1) * 128],
                        h2ph[:, 1 + y0:1 + y0 + rows, dxp:dxp + 2 * W3:2],
                        start=(dxp == 0), stop=False,
                    )
                    nc.tensor.matmul(
                        ps3[:, 0:n3],
                        w3b_t.ap()[64:128, dxp * 128:(dxp + 1) * 128],
                        h2ph[64:128, y0:y0 + rows, dxp:dxp + 2 * W3:2],
                        start=False, stop=(dxp == 2),
                    )
                h3s = scr.tile([128, G3 * W3], dt.bfloat16, tag="h3scr")
                nc.scalar.activation(h3s[:, 0:n3], ps3[:, 0:n3], AF.Relu,
                                     bias=b3_t.ap()[:],
                                     accum_out=h3sums_t.ap()[:, g:g + 1])
            # ---- GAP for this sample ----
            nc.vector.tensor_reduce(hbar_t.ap()[:, s:s + 1], h3sums_t.ap()[:],
                                    axis=mybir.AxisListType.X, op=ALU.add)
        nc.sync.dma_start(hbar_d.ap()[:], hbar_t.ap()[:])

    nc.compile()
    _CACHE[("nc", repeat)] = nc
    return nc


class TileCtx:
    """ExitStack + TileContext combined context manager."""

    def __init__(self, tile_mod, nc):
        self.tile_mod = tile_mod
        self.nc = nc

    def __enter__(self):
        self.ctx = ExitStack()
        self.tc = self.tile_mod.TileContext(self.nc)
        self.tc.__enter__()
        return self.ctx, self.tc

    def __exit__(self, *exc):
        try:
            self.ctx.close()
        finally:
            return self.tc.__exit__(*exc)


def _host_prepare(x, wc1, bc1, wc2, bc2, wc3, bc3, wl1, bl1, wl2, bl2):
    """Build per-core input maps (im2col'd x + weight layouts, bf16)."""
    xp = np.pad(np.asarray(x, dtype=np.float32), ((0, 0), (0, 0), (1, 1), (1, 1)))
    sN, sC, sH, sW = xp.strides
    # x100[b, c, dy, dx, yo, g] = xp[b, c, 4*yo+dy, 4*g+dx]
    win = np.lib.stride_tricks.as_strided(
        xp, (B, CIN, 5, 5, YO, G), (sN, sC, sH, sW, 4 * sH, 4 * sW))
    x100 = np.ascontiguousarray(win.reshape(B, K1, YO, G)).astype(BF16)

    # conv1 weights: lhsT [100, 128]; m = r*64 + j*32 + co; p = c*25 + dy*5 + dx
    w1l = np.zeros((K1, 128), np.float32)
    for r in range(2):
        for j in range(2):
            for dyp in range(3):
                for dxp in range(3):
                    dy, dx = 2 * r + dyp, 2 * j + dxp
                    for c in range(CIN):
                        p = c * 25 + dy * 5 + dx
                        w1l[p, r * 64 + j * 32 + np.arange(32)] = wc1[:, c, dyp, dxp]

    # conv2 shift weights: [128, 4*64]; partition p = yph*64 + xph*32 + c
    SHIFTS = [(0, 0), (0, -1), (-1, 0), (-1, -1)]
    w2l = np.zeros((128, 4 * 64), np.float32)
    for k, (sy, sx) in enumerate(SHIFTS):
        for yph in range(2):
            for xph in range(2):
                if sy == 0:
                    dyp = 1 if yph == 0 else 2
                elif yph == 1:
                    dyp = 0
                else:
                    continue
                if sx == 0:
                    dxp = 1 if xph == 0 else 2
                elif xph == 1:
                    dxp = 0
                else:
                    continue
                for c in range(32):
                    w2l[yph * 64 + xph * 32 + c, k * 64:(k + 1) * 64] = wc2[:, c, dyp, dxp]

    # conv3: A [128, 3*128] (yph0 -> dy'=1, yph1 -> dy'=2); B [64, 3*128] (dy'=0)
    w3a = np.zeros((128, 3 * 128), np.float32)
    w3b = np.zeros((64, 3 * 128), np.float32)
    for dxp in range(3):
        for c in range(64):
            w3a[c, dxp * 128:(dxp + 1) * 128] = wc3[:, c, 1, dxp]
            w3a[64 + c, dxp * 128:(dxp + 1) * 128] = wc3[:, c, 2, dxp]
            w3b[c, dxp * 128:(dxp + 1) * 128] = wc3[:, c, 0, dxp]

    b1 = np.tile(np.asarray(bc1, np.float32), 4).reshape(128, 1)
    b2 = np.tile(np.asarray(bc2, np.float32), 2).reshape(128, 1)
    b3 = np.asarray(bc3, np.float32).reshape(128, 1)

    shared = {
        "w1l": w1l.astype(BF16), "w2l": w2l.astype(BF16),
        "w3a": w3a.astype(BF16), "w3b": w3b.astype(BF16),
        "b1": b1, "b2": b2, "b3": b3,
    }
    in_maps = []
    for core in range(NCORES):
        m = dict(shared)
        m["x100"] = np.ascontiguousarray(x100[core * SPC:(core + 1) * SPC])
        in_maps.append(m)
    return in_maps


def _procrustes(r):
    R = r.reshape(-1, 3, 3).astype(np.float32)
    U, _, Vh = np.linalg.svd(R)
    det = np.linalg.det(U @ Vh)
    U[:, :, -1] *= np.sign(det)[:, None]
    return (U @ Vh).astype(np.float32)


def _host_tail(hbar, wl1, bl1, wl2, bl2):
    """hbar: [B, 128] pooled sums (not yet divided by POOLN)."""
    h = hbar.astype(np.float32) / float(POOLN)
    h = np.maximum(h @ np.asarray(wl1, np.float32).T + np.asarray(bl1, np.float32), 0)
    r = h @ np.asarray(wl2, np.float32).T + np.asarray(bl2, np.float32)
    return _procrustes(r)


def kernel(**inputs):
    from concourse.bass_utils import run_bass_kernel_spmd
    nc = _build_device()
    in_maps = _host_prepare(**inputs)
    res = run_bass_kernel_spmd(nc, in_maps, list(range(NCORES)))
    hbar = np.concatenate(
        [res.results[i]["hbar_out"].T for i in range(NCORES)], axis=0)
    return _host_tail(hbar, inputs["wl1"], inputs["bl1"], inputs["wl2"], inputs["bl2"])


if __name__ == "__main__":
    d = np.load("inputs.npz")
    out = kernel(**{k: d[k] for k in d.files})
    exp = np.load("expected.npy")
    err = np.abs(out - exp).max()
    print("absmax err:", err, "rel:", err / np.abs(exp).max())
